# revision 14
# baseline (speedup 1.0000x reference)
"""Trainium2 Bass kernel for multi-head attention (B=4, S=2048, D=1024, H=16).

Sharding: 8 cores = 4-way batch x 2-way head-group (8 heads per core).
Each core computes, for its batch element b and head-group g:
  qT/kT = (W.T x.T) in transposed layout [local_hd, S] (head-pair chunks of 128
  partitions), V in normal layout [S, local_hd], scores^T = kT.T-free matmuls
  with row-packed head pairs (K=64 x2 concurrent), exp on ACT (no max
  subtraction; scores are provably O(1) for this problem), AV + denominator
  matmuls col-packed via tile_position, reciprocal_approx_fast, normalization
  fused into the PSUM->SBUF copy, then the output projection producing a
  partial [S, D] that the host sums across the 2 head-group cores + bias.
"""

import numpy as np
import ml_dtypes
from contextlib import ExitStack

BF16 = ml_dtypes.bfloat16

S = 2048          # sequence length
D = 1024          # model dim
DH = 64           # head dim
HL = 8            # local heads per core
HD = HL * DH      # 512 local output dims per core
NB = 4            # batch
SCALE = 1.0 / (DH ** 0.5)

KC = D // 128     # 8 contraction chunks for projections
MC = HD // 128    # 4 output-dim chunks (= head pairs) per core
IC = S // 512     # 4 query chunks of 512
JC = S // 128     # 16 key chunks of 128
SC = S // 128     # 16 output row chunks

_NC_CACHE = {}


def _patch_ldw_opt():
    """Enable walrus's LDWEIGHTS optimization (background weight-buffer
    overlap) — bass_utils hardcodes --enable-ldw-opt=false."""
    import os
    if os.environ.get("KLDW", "0") != "1":
        return
    import concourse.bass_utils as _bu
    if getattr(_bu, "_ldw_patched", False):
        return
    _orig = _bu.run_command

    def _patched(argv, **kw):
        argv = ["--enable-ldw-opt=true" if a == "--enable-ldw-opt=false" else a
                for a in argv]
        return _orig(argv, **kw)

    _bu.run_command = _patched
    _bu._ldw_patched = True


def _build_nc():
    import concourse.bacc as bacc
    import concourse.tile as tile
    from concourse import mybir

    _patch_ldw_opt()

    f32 = mybir.dt.float32
    bf16 = mybir.dt.bfloat16
    Exp = mybir.ActivationFunctionType.Exp

    nc = bacc.Bacc("TRN2", target_bir_lowering=False, debug=False)

    xT_d = nc.dram_tensor("xT", [D, S], bf16, kind="ExternalInput")
    wqT_d = nc.dram_tensor("wqT", [D, HD], bf16, kind="ExternalInput")
    wkT_d = nc.dram_tensor("wkT", [D, HD], bf16, kind="ExternalInput")
    wvT_d = nc.dram_tensor("wvT", [D, HD], bf16, kind="ExternalInput")
    woT_d = nc.dram_tensor("woT", [HD, D], bf16, kind="ExternalInput")
    bq_d = nc.dram_tensor("bq", [128, MC], f32, kind="ExternalInput")
    bk_d = nc.dram_tensor("bk", [128, MC], f32, kind="ExternalInput")
    bv_d = nc.dram_tensor("bv", [1, HD], f32, kind="ExternalInput")
    out_d = nc.dram_tensor("out", [S, D], f32, kind="ExternalOutput")

    with tile.TileContext(nc) as tc, ExitStack() as ctx:
        import concourse.bass as bass

        consts = ctx.enter_context(tc.tile_pool(name="consts", bufs=1))
        persist = ctx.enter_context(tc.tile_pool(name="persist", bufs=1))

        woT_sb = consts.tile([128, MC, D], bf16)
        for k in range(MC):
            nc.sync.dma_start(woT_sb[:, k, :], woT_d.ap()[k * 128:(k + 1) * 128, :])

        qT_sb = persist.tile([128, MC, S], bf16)
        kT_sb = persist.tile([128, MC, S], bf16)
        # V layout per (key-chunk, local head): a 128-col block. Even local
        # heads store [V_h(64) | ones(64)], odd heads [ones(64) | V_h(64)].
        # The AV matmul lhsT is then one contiguous block and one matmul
        # produces both the attention output rows and replicated softmax
        # denominator rows.
        v_m = persist.tile([128, JC, HL, 128], bf16)
        avT_sb = persist.tile([128, MC, S], bf16)

        # ---- phases 1+2 fused: projections feed attention; q/k matmuls for
        # pair m+1 are interleaved between attention(m) score groups so the
        # PE has independent work while ACT drains exp backlogs. All
        # projection psum tiles share the "st" pool slots. ----
        with tc.tile_pool(name="xw1", bufs=1) as xw1, \
             tc.tile_pool(name="st", bufs=2, space="PSUM") as stp, \
             tc.tile_pool(name="av", bufs=2, space="PSUM") as avp, \
             tc.tile_pool(name="ep", bufs=4) as ep, \
             tc.tile_pool(name="rp", bufs=4) as rp:
            xT_sb = xw1.tile([128, KC, S], bf16)
            wqT_sb = xw1.tile([128, KC, HD], bf16)
            wkT_sb = xw1.tile([128, KC, HD], bf16)
            wvT_sb = xw1.tile([128, KC, HD], bf16)
            bq_sb = xw1.tile([128, MC], f32)
            bk_sb = xw1.tile([128, MC], f32)
            bvb_sb = xw1.tile([128, HD], f32)  # bv broadcast across partitions

            for k in range(KC):
                nc.sync.dma_start(xT_sb[:, k, :],
                                  xT_d.ap()[k * 128:(k + 1) * 128, :])
                nc.sync.dma_start(wqT_sb[:, k, :],
                                  wqT_d.ap()[k * 128:(k + 1) * 128, :])
                nc.sync.dma_start(wkT_sb[:, k, :],
                                  wkT_d.ap()[k * 128:(k + 1) * 128, :])
                nc.sync.dma_start(wvT_sb[:, k, :],
                                  wvT_d.ap()[k * 128:(k + 1) * 128, :])
            nc.sync.dma_start(bq_sb[:], bq_d.ap())
            nc.sync.dma_start(bk_sb[:], bk_d.ap())
            # broadcast bv along partitions via 0-step AP
            bv_ap = bv_d.ap()
            bv_bcast = bass.AP(tensor=bv_ap.tensor, offset=bv_ap.offset,
                               ap=[[0, 128]] + [bv_ap.ap[-1]])
            nc.sync.dma_start(bvb_sb[:], bv_bcast)

            nc.vector.memset(v_m[:], 1.0)  # ones blocks; V overwrites its own
            bvb_r = bvb_sb[:].rearrange("p (h e) -> p h e", h=HL)

            def v_unit(t):
                # V in normal layout [S, local_hd]: lhsT = x^T chunk, rhs=wv^T
                tsl = slice(t * 128, (t + 1) * 128)
                psv = stp.tile([128, 3, 512], f32, tag="st")
                for k in range(KC):
                    nc.tensor.matmul(psv[:, 0], xT_sb[:, k, tsl],
                                     wvT_sb[:, k, :],
                                     start=(k == 0), stop=(k == KC - 1))
                psv_r = psv[:, 0].rearrange("p (h e) -> p h e", h=HL)
                # even heads -> cols 0-63 of their block, odd -> cols 64-127
                nc.vector.tensor_add(v_m[:, t, 0::2, 0:64],
                                     psv_r[:, 0::2, :], bvb_r[:, 0::2, :])
                nc.vector.tensor_add(v_m[:, t, 1::2, 64:128],
                                     psv_r[:, 1::2, :], bvb_r[:, 1::2, :])

            def qk_unit(m, which, i):
                # qT/kT in [local_hd, S]: lhsT = W^T chunk (stationary)
                w_sb, b_sb, dst = ((wqT_sb, bq_sb, qT_sb) if which == 0
                                   else (wkT_sb, bk_sb, kT_sb))
                isl = slice(i * 512, (i + 1) * 512)
                msl = slice(m * 128, (m + 1) * 128)
                ps = stp.tile([128, 3, 512], f32, tag="st")
                for k in range(KC):
                    nc.tensor.matmul(ps[:, 0], w_sb[:, k, msl],
                                     xT_sb[:, k, isl],
                                     start=(k == 0), stop=(k == KC - 1))
                nc.vector.tensor_scalar_add(dst[:, m, isl], ps[:, 0],
                                            b_sb[:, m:m + 1])

            for t in range(JC):
                v_unit(t)
            for which in range(2):
                for i in range(IC):
                    qk_unit(0, which, i)

            for m in range(MC):
                h0 = 2 * m
                # q/k units for the next pair, doled out between score groups
                next_units = ([(m + 1, w, i) for w in range(2)
                               for i in range(IC)] if m + 1 < MC else [])
                ucursor = 0
                gcount = 0
                for i in range(IC):
                    isl = slice(i * 512, (i + 1) * 512)
                    avh = avp.tile([128, 512], f32, tag="av")
                    avh1 = avp.tile([128, 512], f32, tag="av")
                    # j-range in 2 blocks; per block, loop A does scores+exp
                    # (PE in 64x128 row-tiled mode: head h on row tile T0,
                    # h+1 on T8), loop B accumulates AV+denominator in plain
                    # 128x128 mode. Batching same-mode matmuls avoids the
                    # per-switch TensorE drain.
                    for jb in range(2):
                        e_h = ep.tile([128, JC // 2 * 512], bf16, tag="e")
                        e_h1 = ep.tile([128, JC // 2 * 512], bf16, tag="e")
                        eoff = 0
                        ljs = list(range(jb * (JC // 2), (jb + 1) * (JC // 2)))
                        # exp groups of 3 PSUM banks (FD=1536) amortize the
                        # per-op ACT overhead
                        groups = [ljs[0:3], ljs[3:6], ljs[6:8]]
                        for grp in groups:
                            g = len(grp)
                            st_h = stp.tile([128, 3, 512], f32, tag="st")
                            st_h1 = stp.tile([128, 3, 512], f32, tag="st")
                            for gi, j in enumerate(grp):
                                jsl = slice(j * 128, (j + 1) * 128)
                                nc.tensor.matmul(st_h[:, gi],
                                                 kT_sb[0:64, m, jsl],
                                                 qT_sb[0:64, m, isl],
                                                 start=True, stop=True)
                                nc.tensor.matmul(st_h1[:, gi],
                                                 kT_sb[64:128, m, jsl],
                                                 qT_sb[64:128, m, isl],
                                                 start=True, stop=True)
                            esl = slice(eoff, eoff + g * 512)
                            nc.scalar.activation(e_h[:, esl], st_h[:, 0:g], Exp)
                            nc.scalar.activation(e_h1[:, esl], st_h1[:, 0:g],
                                                 Exp)
                            eoff += g * 512
                            # every 3rd group, give the PE a projection unit
                            gcount += 1
                            if gcount % 3 == 0 and ucursor < len(next_units):
                                qk_unit(*next_units[ucursor])
                                ucursor += 1
                        for lj in range(JC // 2):
                            j = jb * (JC // 2) + lj
                            jsl = slice(lj * 512, (lj + 1) * 512)
                            first = (j == 0)
                            last = (j == JC - 1)
                            # head h: [V_h | ones] -> U at rows 0-63,
                            # denominator replicated at rows 64-127
                            nc.tensor.matmul(avh[:], v_m[:, j, h0, :],
                                             e_h[:, jsl],
                                             start=first, stop=last)
                            # head h+1: [ones | V_h1] -> denominator at
                            # rows 0-63, U at rows 64-127
                            nc.tensor.matmul(avh1[:], v_m[:, j, h0 + 1, :],
                                             e_h1[:, jsl],
                                             start=first, stop=last)
                    # epilogue: gather denominators (lane-aligned copies),
                    # reciprocal, partition-swap halves via SBUF->SBUF DMA,
                    # then normalize fused into the PSUM->SBUF copy.
                    dcomb = rp.tile([128, 512], f32, tag="r")
                    nc.vector.tensor_copy(dcomb[64:128, :], avh[64:128, :])
                    nc.vector.tensor_copy(dcomb[0:64, :], avh1[0:64, :])
                    rcomb = rp.tile([128, 512], f32, tag="r")
                    nc.vector.reciprocal_approx_fast(out=rcomb[:],
                                                     in_=dcomb[:])
                    rswap = rp.tile([128, 512], f32, tag="r")
                    nc.sync.dma_start(rswap[0:64, :], rcomb[64:128, :])
                    nc.sync.dma_start(rswap[64:128, :], rcomb[0:64, :])
                    nc.vector.tensor_mul(avT_sb[0:64, m, isl], avh[0:64, :],
                                         rswap[0:64, :])
                    nc.vector.tensor_mul(avT_sb[64:128, m, isl],
                                         avh1[64:128, :], rswap[64:128, :])
                # any leftover next-pair units
                while ucursor < len(next_units):
                    qk_unit(*next_units[ucursor])
                    ucursor += 1

        # ---- phase 3: output projection (partial; host sums over 2 cores) ----
        with tc.tile_pool(name="ps3", bufs=2, space="PSUM") as ps3, \
             tc.tile_pool(name="og", bufs=3) as ogp:
            for sc in range(SC):
                ssl = slice(sc * 128, (sc + 1) * 128)
                po = ps3.tile([128, 2, 512], f32, tag="po")
                for k2 in range(MC):
                    for nh in range(2):
                        nc.tensor.matmul(po[:, nh], avT_sb[:, k2, ssl],
                                         woT_sb[:, k2, nh * 512:(nh + 1) * 512],
                                         start=(k2 == 0), stop=(k2 == MC - 1))
                og = ogp.tile([128, D], f32, tag="og")
                nc.vector.tensor_copy(og[:], po[:])
                nc.sync.dma_start(out_d.ap()[ssl, :], og[:])

    nc.compile()
    return nc


def _get_nc():
    if "nc" not in _NC_CACHE:
        _NC_CACHE["nc"] = _build_nc()
    return _NC_CACHE["nc"]


def kernel(x, Wq, bq, Wk, bk, Wv, bv, Wo, bo):
    from concourse.bass_utils import run_bass_kernel_spmd

    x = np.asarray(x, dtype=np.float32)
    Wq = np.asarray(Wq, dtype=np.float32)
    Wk = np.asarray(Wk, dtype=np.float32)
    Wv = np.asarray(Wv, dtype=np.float32)
    Wo = np.asarray(Wo, dtype=np.float32)
    bq = np.asarray(bq, dtype=np.float32)
    bk = np.asarray(bk, dtype=np.float32)
    bv = np.asarray(bv, dtype=np.float32)
    bo = np.asarray(bo, dtype=np.float32)

    nc = _get_nc()

    in_maps = []
    for c in range(8):
        b = c // 2
        g = c % 2
        sl = slice(g * HD, (g + 1) * HD)
        in_maps.append({
            "xT": np.ascontiguousarray(x[b].T).astype(BF16),
            "wqT": np.ascontiguousarray((Wq[sl] * SCALE).T).astype(BF16),
            "wkT": np.ascontiguousarray(Wk[sl].T).astype(BF16),
            "wvT": np.ascontiguousarray(Wv[sl].T).astype(BF16),
            "woT": np.ascontiguousarray(Wo[:, sl].T).astype(BF16),
            "bq": np.ascontiguousarray((bq[sl] * SCALE).reshape(MC, 128).T),
            "bk": np.ascontiguousarray(bk[sl].reshape(MC, 128).T),
            "bv": bv[sl].reshape(1, HD).astype(np.float32),
        })

    _NC_CACHE["last_in_maps"] = in_maps
    res = run_bass_kernel_spmd(nc, in_maps, core_ids=list(range(8)))
    outs = [res.results[c]["out"] for c in range(8)]
    out = np.stack([outs[2 * b] + outs[2 * b + 1] for b in range(NB)])
    out = out + bo[None, None, :]
    return out.astype(np.float32)


# revision 21
# speedup vs baseline: 1.1384x; 1.1384x over previous
"""Trainium2 Bass kernel for multi-head attention (B=4, S=2048, D=1024, H=16).

Sharding: 8 cores = 4-way batch x 2-way head-group (8 heads per core).
Each core computes, for its batch element b and head-group g:
  qT/kT = (W.T x.T) in transposed layout [local_hd, S] (head-pair chunks of 128
  partitions), V in normal layout [S, local_hd], scores^T = kT.T-free matmuls
  with row-packed head pairs (K=64 x2 concurrent), exp on ACT (no max
  subtraction; scores are provably O(1) for this problem), AV + denominator
  matmuls col-packed via tile_position, reciprocal_approx_fast, normalization
  fused into the PSUM->SBUF copy, then the output projection producing a
  partial [S, D] that the host sums across the 2 head-group cores + bias.
"""

import numpy as np
import ml_dtypes
from contextlib import ExitStack

BF16 = ml_dtypes.bfloat16

S = 2048          # sequence length
D = 1024          # model dim
DH = 64           # head dim
HL = 8            # local heads per core
HD = HL * DH      # 512 local output dims per core
NB = 4            # batch
SCALE = 1.0 / (DH ** 0.5)

KC = D // 128     # 8 contraction chunks for projections
MC = HD // 128    # 4 output-dim chunks (= head pairs) per core
IC = S // 512     # 4 query chunks of 512
JC = S // 128     # 16 key chunks of 128
SC = S // 128     # 16 output row chunks

_NC_CACHE = {}


def _patch_ldw_opt():
    """Enable walrus's LDWEIGHTS optimization (background weight-buffer
    overlap) — bass_utils hardcodes --enable-ldw-opt=false."""
    import os
    if os.environ.get("KLDW", "0") != "1":
        return
    import concourse.bass_utils as _bu
    if getattr(_bu, "_ldw_patched", False):
        return
    _orig = _bu.run_command

    def _patched(argv, **kw):
        argv = ["--enable-ldw-opt=true" if a == "--enable-ldw-opt=false" else a
                for a in argv]
        return _orig(argv, **kw)

    _bu.run_command = _patched
    _bu._ldw_patched = True


def _build_nc():
    import concourse.bacc as bacc
    import concourse.tile as tile
    from concourse import mybir

    _patch_ldw_opt()

    f32 = mybir.dt.float32
    bf16 = mybir.dt.bfloat16
    Exp = mybir.ActivationFunctionType.Exp

    nc = bacc.Bacc("TRN2", target_bir_lowering=False, debug=False)

    xT_d = nc.dram_tensor("xT", [D, S], bf16, kind="ExternalInput")
    wqT_d = nc.dram_tensor("wqT", [D, HD], bf16, kind="ExternalInput")
    wkT_d = nc.dram_tensor("wkT", [D, HD], bf16, kind="ExternalInput")
    wvT_d = nc.dram_tensor("wvT", [D, HD], bf16, kind="ExternalInput")
    woT_d = nc.dram_tensor("woT", [HD, D], bf16, kind="ExternalInput")
    bq_d = nc.dram_tensor("bq", [128, MC], f32, kind="ExternalInput")
    bk_d = nc.dram_tensor("bk", [128, MC], f32, kind="ExternalInput")
    bv_d = nc.dram_tensor("bv", [1, HD], f32, kind="ExternalInput")
    out_d = nc.dram_tensor("out", [S, D], f32, kind="ExternalOutput")

    with tile.TileContext(nc) as tc, ExitStack() as ctx:
        import concourse.bass as bass

        consts = ctx.enter_context(tc.tile_pool(name="consts", bufs=1))
        persist = ctx.enter_context(tc.tile_pool(name="persist", bufs=1))

        woT_sb = consts.tile([128, MC, D], bf16)
        for k in range(MC):
            nc.sync.dma_start(woT_sb[:, k, :], woT_d.ap()[k * 128:(k + 1) * 128, :])

        qT_sb = persist.tile([128, MC, S], bf16)
        kT_sb = persist.tile([128, MC, S], bf16)
        # V layout per (key-chunk, local head): a 128-col block. Even local
        # heads store [V_h(64) | ones(64)], odd heads [ones(64) | V_h(64)].
        # The AV matmul lhsT is then one contiguous block and one matmul
        # produces both the attention output rows and replicated softmax
        # denominator rows.
        v_m = persist.tile([128, JC, HL, 128], bf16)
        avT_sb = persist.tile([128, MC, S], bf16)

        # ---- phases 1+2 fused: projections feed attention; q/k matmuls for
        # pair m+1 are interleaved between attention(m) score groups so the
        # PE has independent work while ACT drains exp backlogs. All
        # projection psum tiles share the "st" pool slots. ----
        with tc.tile_pool(name="xw1", bufs=1) as xw1, \
             tc.tile_pool(name="st", bufs=2, space="PSUM") as stp, \
             tc.tile_pool(name="av", bufs=2, space="PSUM") as avp, \
             tc.tile_pool(name="ep", bufs=4) as ep, \
             tc.tile_pool(name="sgp", bufs=4) as sgp, \
             tc.tile_pool(name="rp", bufs=4) as rp:
            xT_sb = xw1.tile([128, KC, S], bf16)
            wqT_sb = xw1.tile([128, KC, HD], bf16)
            wkT_sb = xw1.tile([128, KC, HD], bf16)
            wvT_sb = xw1.tile([128, KC, HD], bf16)
            bq_sb = xw1.tile([128, MC], f32)
            bk_sb = xw1.tile([128, MC], f32)
            bvb_sb = xw1.tile([128, HD], f32)  # bv broadcast across partitions

            for k in range(KC):
                nc.sync.dma_start(xT_sb[:, k, :],
                                  xT_d.ap()[k * 128:(k + 1) * 128, :])
            for k in range(KC):
                nc.sync.dma_start(wvT_sb[:, k, :],
                                  wvT_d.ap()[k * 128:(k + 1) * 128, :])
            for k in range(KC):
                nc.sync.dma_start(wqT_sb[:, k, :],
                                  wqT_d.ap()[k * 128:(k + 1) * 128, :])
                nc.sync.dma_start(wkT_sb[:, k, :],
                                  wkT_d.ap()[k * 128:(k + 1) * 128, :])
            nc.sync.dma_start(bq_sb[:], bq_d.ap())
            nc.sync.dma_start(bk_sb[:], bk_d.ap())
            # broadcast bv along partitions via 0-step AP
            bv_ap = bv_d.ap()
            bv_bcast = bass.AP(tensor=bv_ap.tensor, offset=bv_ap.offset,
                               ap=[[0, 128]] + [bv_ap.ap[-1]])
            nc.sync.dma_start(bvb_sb[:], bv_bcast)

            nc.vector.memset(v_m[:], 1.0)  # ones blocks; V overwrites its own
            bvb_r = bvb_sb[:].rearrange("p (h e) -> p h e", h=HL)

            def v_unit(t):
                # V in normal layout [S, local_hd]: lhsT = x^T chunk, rhs=wv^T
                tsl = slice(t * 128, (t + 1) * 128)
                psv = stp.tile([128, 3, 512], f32, tag="st")
                for k in range(KC):
                    nc.tensor.matmul(psv[:, 0], xT_sb[:, k, tsl],
                                     wvT_sb[:, k, :],
                                     start=(k == 0), stop=(k == KC - 1))
                psv_r = psv[:, 0].rearrange("p (h e) -> p h e", h=HL)
                # even heads -> cols 0-63 of their block, odd -> cols 64-127
                nc.vector.tensor_add(v_m[:, t, 0::2, 0:64],
                                     psv_r[:, 0::2, :], bvb_r[:, 0::2, :])
                nc.vector.tensor_add(v_m[:, t, 1::2, 64:128],
                                     psv_r[:, 1::2, :], bvb_r[:, 1::2, :])

            def qk_unit(m, which, i):
                # qT/kT in [local_hd, S]: lhsT = W^T chunk (stationary)
                w_sb, b_sb, dst = ((wqT_sb, bq_sb, qT_sb) if which == 0
                                   else (wkT_sb, bk_sb, kT_sb))
                isl = slice(i * 512, (i + 1) * 512)
                msl = slice(m * 128, (m + 1) * 128)
                ps = stp.tile([128, 3, 512], f32, tag="st")
                for k in range(KC):
                    nc.tensor.matmul(ps[:, 0], w_sb[:, k, msl],
                                     xT_sb[:, k, isl],
                                     start=(k == 0), stop=(k == KC - 1))
                nc.vector.tensor_scalar_add(dst[:, m, isl], ps[:, 0],
                                            b_sb[:, m:m + 1])

            for t in range(JC):
                v_unit(t)
            for mm in range(MC):
                for which in range(2):
                    for i in range(IC):
                        qk_unit(mm, which, i)

            for m in range(MC):
                h0 = 2 * m
                for i in range(IC):
                    isl = slice(i * 512, (i + 1) * 512)
                    avh = avp.tile([128, 512], f32, tag="av")
                    avh1 = avp.tile([128, 512], f32, tag="av")
                    # j-range in 2 blocks; per block, loop A does scores+exp
                    # (PE in 64x128 row-tiled mode: head h on row tile T0,
                    # h+1 on T8), loop B accumulates AV+denominator in plain
                    # 128x128 mode. Batching same-mode matmuls avoids the
                    # per-switch TensorE drain.
                    for jb in range(2):
                        e_h = ep.tile([128, JC // 2 * 512], bf16, tag="e")
                        e_h1 = ep.tile([128, JC // 2 * 512], bf16, tag="e")
                        eoff = 0
                        ljs = list(range(jb * (JC // 2), (jb + 1) * (JC // 2)))
                        # exp groups of 3 PSUM banks (FD=1536) amortize the
                        # per-op ACT overhead
                        groups = [ljs[0:3], ljs[3:6], ljs[6:8]]
                        for grp in groups:
                            g = len(grp)
                            st_h = stp.tile([128, 3, 512], f32, tag="st")
                            st_h1 = stp.tile([128, 3, 512], f32, tag="st")
                            for gi, j in enumerate(grp):
                                jsl = slice(j * 128, (j + 1) * 128)
                                nc.tensor.matmul(st_h[:, gi],
                                                 kT_sb[0:64, m, jsl],
                                                 qT_sb[0:64, m, isl],
                                                 start=True, stop=True)
                                nc.tensor.matmul(st_h1[:, gi],
                                                 kT_sb[64:128, m, jsl],
                                                 qT_sb[64:128, m, isl],
                                                 start=True, stop=True)
                            esl = slice(eoff, eoff + g * 512)
                            nc.scalar.activation(e_h[:, esl], st_h[:, 0:g], Exp)
                            nc.scalar.activation(e_h1[:, esl], st_h1[:, 0:g],
                                                 Exp)
                            eoff += g * 512
                        for lj in range(JC // 2):
                            j = jb * (JC // 2) + lj
                            jsl = slice(lj * 512, (lj + 1) * 512)
                            first = (j == 0)
                            last = (j == JC - 1)
                            # head h: [V_h | ones] -> U at rows 0-63,
                            # denominator replicated at rows 64-127
                            nc.tensor.matmul(avh[:], v_m[:, j, h0, :],
                                             e_h[:, jsl],
                                             start=first, stop=last)
                            # head h+1: [ones | V_h1] -> denominator at
                            # rows 0-63, U at rows 64-127
                            nc.tensor.matmul(avh1[:], v_m[:, j, h0 + 1, :],
                                             e_h1[:, jsl],
                                             start=first, stop=last)
                    # epilogue: stage PSUM->SBUF immediately so the AV banks
                    # free after two fast copies; reciprocal + partition-swap
                    # (SBUF->SBUF DMA) + normalize run on the staging copies.
                    sg_h = sgp.tile([128, 512], f32, tag="sg")
                    sg_h1 = sgp.tile([128, 512], f32, tag="sg")
                    nc.vector.tensor_copy(sg_h[:], avh[:])
                    nc.vector.tensor_copy(sg_h1[:], avh1[:])
                    dcomb = rp.tile([128, 512], f32, tag="r")
                    nc.vector.tensor_copy(dcomb[64:128, :], sg_h[64:128, :])
                    nc.vector.tensor_copy(dcomb[0:64, :], sg_h1[0:64, :])
                    rcomb = rp.tile([128, 512], f32, tag="r")
                    nc.vector.reciprocal_approx_fast(out=rcomb[:],
                                                     in_=dcomb[:])
                    rswap = rp.tile([128, 512], f32, tag="r")
                    nc.sync.dma_start(rswap[0:64, :], rcomb[64:128, :])
                    nc.sync.dma_start(rswap[64:128, :], rcomb[0:64, :])
                    nc.vector.tensor_mul(avT_sb[0:64, m, isl], sg_h[0:64, :],
                                         rswap[0:64, :])
                    nc.vector.tensor_mul(avT_sb[64:128, m, isl],
                                         sg_h1[64:128, :], rswap[64:128, :])

        # ---- phase 3: output projection (partial; host sums over 2 cores) ----
        with tc.tile_pool(name="ps3", bufs=2, space="PSUM") as ps3, \
             tc.tile_pool(name="og", bufs=2) as ogp:
            for sc in range(SC):
                ssl = slice(sc * 128, (sc + 1) * 128)
                po = ps3.tile([128, 2, 512], f32, tag="po")
                for k2 in range(MC):
                    for nh in range(2):
                        nc.tensor.matmul(po[:, nh], avT_sb[:, k2, ssl],
                                         woT_sb[:, k2, nh * 512:(nh + 1) * 512],
                                         start=(k2 == 0), stop=(k2 == MC - 1))
                og = ogp.tile([128, D], f32, tag="og")
                nc.vector.tensor_copy(og[:], po[:])
                nc.sync.dma_start(out_d.ap()[ssl, :], og[:])

    nc.compile()
    return nc


def _get_nc():
    if "nc" not in _NC_CACHE:
        _NC_CACHE["nc"] = _build_nc()
    return _NC_CACHE["nc"]


def kernel(x, Wq, bq, Wk, bk, Wv, bv, Wo, bo):
    from concourse.bass_utils import run_bass_kernel_spmd

    x = np.asarray(x, dtype=np.float32)
    Wq = np.asarray(Wq, dtype=np.float32)
    Wk = np.asarray(Wk, dtype=np.float32)
    Wv = np.asarray(Wv, dtype=np.float32)
    Wo = np.asarray(Wo, dtype=np.float32)
    bq = np.asarray(bq, dtype=np.float32)
    bk = np.asarray(bk, dtype=np.float32)
    bv = np.asarray(bv, dtype=np.float32)
    bo = np.asarray(bo, dtype=np.float32)

    nc = _get_nc()

    in_maps = []
    for c in range(8):
        b = c // 2
        g = c % 2
        sl = slice(g * HD, (g + 1) * HD)
        in_maps.append({
            "xT": np.ascontiguousarray(x[b].T).astype(BF16),
            "wqT": np.ascontiguousarray((Wq[sl] * SCALE).T).astype(BF16),
            "wkT": np.ascontiguousarray(Wk[sl].T).astype(BF16),
            "wvT": np.ascontiguousarray(Wv[sl].T).astype(BF16),
            "woT": np.ascontiguousarray(Wo[:, sl].T).astype(BF16),
            "bq": np.ascontiguousarray((bq[sl] * SCALE).reshape(MC, 128).T),
            "bk": np.ascontiguousarray(bk[sl].reshape(MC, 128).T),
            "bv": bv[sl].reshape(1, HD).astype(np.float32),
        })

    _NC_CACHE["last_in_maps"] = in_maps
    res = run_bass_kernel_spmd(nc, in_maps, core_ids=list(range(8)))
    outs = [res.results[c]["out"] for c in range(8)]
    out = np.stack([outs[2 * b] + outs[2 * b + 1] for b in range(NB)])
    out = out + bo[None, None, :]
    return out.astype(np.float32)
